# revision 1
# baseline (speedup 1.0000x reference)
"""EdgeCrossingsLoss Trainium2 kernel (8-core SPMD, data-parallel over query faces).

Two device launches (this bedrock runtime ships no Q7 extended-instruction
ucode, so there is no usable on-device gather; the host does the small
index-merge + geometry gather between the launches):

prog1 (per core, 1280 query rows = 10 tiles of 128):
  PE:  -d2[q, c] = 2*bary_q.bary_c - sq_q - sq_c for all 10240 candidates via a
       K=16 bf16 hi/lo-split matmul (bf16 products are exact, accumulated in
       f32 PSUM -> f32-quality d2). rhs sits in four 16-partition bands at
       base partitions 0/32/64/96 (PE row-tiles) so its DMA is wide.
  ACT: copies each PSUM block into a linear [128, 10240] SBUF -d2 row block.
  DVE: per 2560-chunk, max8 (top-8 values) + max_index (in-chunk positions).
       Output [128, 32] values + indices per tile.

host: exact top-16 merge of the 4 chunk-top-8s per row (lexsort by value desc /
      index asc = the jax top_k tie-break). Rows where a chunk's reported 8
      values all rank above our 16th (the chunk could hide a 9th member of the
      true top-16) are recomputed exactly on the host (vectorized, ~10% of
      rows). Gathers the 16 neighbor faces' edge geometry; folds probabilities
      and the self-neighbor mask into per-(row, slot) weights.

prog2 (per core): all 1280x16 3x3 line-line crossing tests in one batch of
      broadcast-AP tensor ops on DVE (Pool rejects broadcast APs, ACT
      replicates the query geometry), hit = num^2 < EPS^2*|cross|^2 (den=0 /
      NaN cases fall out correctly), weight-masked and reduced per row.

Host sums the 8 per-core partials and divides by num_faces.
"""
import os
import numpy as np
import ml_dtypes
from contextlib import ExitStack

import concourse.bass as bass
import concourse.tile as tile
import concourse.bacc as bacc
from concourse import mybir
from concourse.bass_utils import run_bass_kernel_spmd

F32 = mybir.dt.float32
BF16 = mybir.dt.bfloat16
U16 = mybir.dt.uint16

NCORES = 8
KNN = 16
EPS = 1e-5
FP = 10240            # padded candidate count
NR = FP // NCORES     # 1280 rows per core
NT = NR // 128        # 10 tiles of 128 rows
KMM = 16              # matmul contraction rows (bf16 hi/lo split)
NGRP = 4              # rhs partition bands (at partitions 0/32/64/96)
GW = FP // NGRP       # 2560
PSW = GW // 2         # 1280-wide PSUM tiles (3 banks)
MMCH = 512            # matmul N per instruction (one PSUM bank)
MXCH = 2560           # max8/max_index chunk in SBUF
NCH = FP // MXCH      # 4 chunks
NC8 = NCH * 8         # 40 chunk-top-8 candidates per row
GPS = 10              # prog2: slots [0:GPS) on DVE, [GPS:16) on GPSIMD

ALU = mybir.AluOpType


def _build_prog1():
    nc = bacc.Bacc("TRN2", target_bir_lowering=False, debug=False,
                   num_devices=NCORES)
    # band b occupies partitions [32b, 32b+16); lhsT replicated into each band
    lhsT_in = nc.dram_tensor("lhsT", [128, NR], BF16, kind="ExternalInput").ap()
    rhs_in = nc.dram_tensor("rhs", [128, GW], BF16, kind="ExternalInput").ap()
    cv_out = nc.dram_tensor("cv", [NT, 128, NC8], F32, kind="ExternalOutput").ap()
    ci_out = nc.dram_tensor("ci", [NT, 128, NC8], U16, kind="ExternalOutput").ap()

    with tile.TileContext(nc) as tc, ExitStack() as ctx:
        const_pool = ctx.enter_context(tc.tile_pool(name="const", bufs=1))
        psum_pool = ctx.enter_context(tc.tile_pool(name="psum", bufs=2, space="PSUM"))
        negd2_pool = ctx.enter_context(tc.tile_pool(name="negd2", bufs=2))
        out_pool = ctx.enter_context(tc.tile_pool(name="out", bufs=2))

        lhsT_sb = const_pool.tile([128, NR], BF16)
        nc.sync.dma_start(lhsT_sb[:], lhsT_in[:])
        rhs_sb = const_pool.tile([128, GW], BF16)
        for j in range(4):   # column chunks on two queues: matmuls start early
            eng = (nc.scalar, nc.sync)[j % 2]
            eng.dma_start(rhs_sb[:, j * (GW // 4):(j + 1) * (GW // 4)],
                          rhs_in[:, j * (GW // 4):(j + 1) * (GW // 4)])

        for t in range(NT):
            negd2 = negd2_pool.tile([128, FP], F32, tag="negd2")
            cv = out_pool.tile([128, NC8], F32, tag="cv")
            ci = out_pool.tile([128, NC8], U16, tag="ci")
            for g in range(NGRP):
                for h in range(GW // PSW):
                    ps = psum_pool.tile([128, PSW], F32, tag="ps")
                    base = h * PSW
                    for c0 in range(base, base + PSW, MMCH):
                        n = min(MMCH, base + PSW - c0)
                        nc.tensor.matmul(
                            ps[:, c0 - base:c0 - base + n],
                            lhsT=lhsT_sb[32 * g:32 * g + KMM,
                                         t * 128:(t + 1) * 128],
                            rhs=rhs_sb[32 * g:32 * g + KMM, c0:c0 + n],
                            start=True, stop=True,
                            tile_position=(32 * g, 0),
                        )
                    nc.scalar.copy(
                        negd2[:, g * GW + base:g * GW + base + PSW], ps[:])
            for m in range(NCH):
                nc.vector.max(cv[:, m * 8:(m + 1) * 8],
                              negd2[:, m * MXCH:(m + 1) * MXCH])
                nc.vector.max_index(ci[:, m * 8:(m + 1) * 8],
                                    cv[:, m * 8:(m + 1) * 8],
                                    negd2[:, m * MXCH:(m + 1) * MXCH])
            nc.sync.dma_start(cv_out[t], cv[:])
            nc.sync.dma_start(ci_out[t], ci[:])

    nc.compile()
    return nc


def _build_prog2():
    nc = bacc.Bacc("TRN2", target_bir_lowering=False, debug=False,
                   num_devices=NCORES)
    # host pre-transposes to partition-major layouts
    geom_in = nc.dram_tensor("geomN", [128, NT, KNN, 18], F32, kind="ExternalInput").ap()
    qgeom_in = nc.dram_tensor("qgeom", [128, NT, 18], F32, kind="ExternalInput").ap()
    vp_in = nc.dram_tensor("vp", [128, NT, KNN], F32, kind="ExternalInput").ap()
    wcross_out = nc.dram_tensor("wcross", [128, NT], F32, kind="ExternalOutput").ap()

    with tile.TileContext(nc) as tc, ExitStack() as ctx:
        pool = ctx.enter_context(tc.tile_pool(name="p", bufs=1))

        TS = NT * KNN
        # small inputs first so the ACT qgr replicate starts immediately;
        # geom as two large half-DMAs on separate HWDGE queues
        nc.sync.dma_start(qg := pool.tile([128, NT, 18], F32, name="qg"),
                          qgeom_in[:])
        nc.scalar.dma_start(vp := pool.tile([128, TS], F32, name="vp"),
                            vp_in[:].rearrange("p t s -> p (t s)"))
        geom = pool.tile([128, TS, 18], F32)
        H = NT // 2
        nc.sync.dma_start(
            geom[:, :H * KNN, :],
            geom_in[:, :H].rearrange("p t s c -> p (t s) c"))
        nc.scalar.dma_start(
            geom[:, H * KNN:, :],
            geom_in[:, H:].rearrange("p t s c -> p (t s) c"))

        # replicate query geometry per neighbor slot (ACT is otherwise idle)
        qgr = pool.tile([128, TS, 18], F32)
        nc.scalar.copy(
            qgr[:].rearrange("p (t s) c -> p t s c", t=NT),
            qg[:].unsqueeze(2).broadcast_to([128, NT, KNN, 18]))

        hit = pool.tile([128, TS, 3, 3], F32)

        def emit(beng, meng, x0, x1):
            """Edge tests for combined (tile, slot) range [x0, x1).
            beng runs the broadcast-AP ops (DVE); meng the unit-stride chain."""
            nx = x1 - x0
            SH = [128, nx, 3, 3]
            xsl = slice(x0, x1)

            def uc(c):   # query edge dir comp c (varies e1)
                return qgr[:, xsl, 9 + c:18:3].unsqueeze(3).broadcast_to(SH)

            def sc(c):   # query edge start comp c
                return qgr[:, xsl, c:9:3].unsqueeze(3).broadcast_to(SH)

            def vc(c):   # neighbor edge dir comp c (varies e2)
                return geom[:, xsl, 9 + c:18:3].unsqueeze(2).broadcast_to(SH)

            def tcp(c):  # neighbor edge start comp c
                return geom[:, xsl, c:9:3].unsqueeze(2).broadcast_to(SH)

            pfx = f"e{x0}"
            m = [pool.tile(SH, F32, name=f"{pfx}_m{i}") for i in range(6)]
            dif = [pool.tile(SH, F32, name=f"{pfx}_d{i}") for i in range(3)]
            cr = [pool.tile(SH, F32, name=f"{pfx}_cr{i}") for i in range(3)]
            BT = beng.tensor_tensor
            MT = meng.tensor_tensor
            for i in range(3):  # cr_i = u_{i+1} * v_{i+2} - u_{i+2} * v_{i+1}
                a, b = (i + 1) % 3, (i + 2) % 3
                BT(m[2 * i][:], uc(a), vc(b), ALU.mult)
                BT(m[2 * i + 1][:], uc(b), vc(a), ALU.mult)
            for c in range(3):
                BT(dif[c][:], tcp(c), sc(c), ALU.subtract)
            for i in range(3):
                MT(cr[i][:], m[2 * i][:], m[2 * i + 1][:], ALU.subtract)

            num = pool.tile(SH, F32, name=f"{pfx}_num")
            den2 = pool.tile(SH, F32, name=f"{pfx}_den2")
            t0 = pool.tile(SH, F32, name=f"{pfx}_t0")
            t1 = pool.tile(SH, F32, name=f"{pfx}_t1")
            MT(num[:], dif[0][:], cr[0][:], ALU.mult)
            MT(den2[:], cr[0][:], cr[0][:], ALU.mult)
            for c in (1, 2):
                MT(t0[:], dif[c][:], cr[c][:], ALU.mult)
                MT(num[:], num[:], t0[:], ALU.add)
                MT(t1[:], cr[c][:], cr[c][:], ALU.mult)
                MT(den2[:], den2[:], t1[:], ALU.add)
            MT(num[:], num[:], num[:], ALU.mult)          # num^2
            meng.tensor_scalar(den2[:], den2[:], float(EPS * EPS), None, ALU.mult)
            h = hit[:, xsl]
            MT(h, num[:], den2[:], ALU.is_lt)             # num^2 < eps^2*|cr|^2
            BT(h, h, vp[:, xsl].unsqueeze(2).unsqueeze(3).broadcast_to(SH),
               ALU.mult)

        emit(nc.vector, nc.vector, 0, TS // 2)
        emit(nc.vector, nc.vector, TS // 2, TS)

        wtile = pool.tile([128, NT], F32)
        nc.vector.tensor_reduce(
            wtile[:], hit[:].rearrange("p (t s) a b -> p t (s a b)", t=NT),
            mybir.AxisListType.X, ALU.add)


        nc.sync.dma_start(wcross_out[:], wtile[:])

    nc.compile()
    return nc


_PROGS = {}


def _get_progs():
    if "p1" not in _PROGS:
        _PROGS["p1"] = _build_prog1()
        _PROGS["p2"] = _build_prog2()
    return _PROGS["p1"], _PROGS["p2"]


def _host_prep(vertices, faces, probabilities):
    V = np.ascontiguousarray(vertices, dtype=np.float32)
    Fc = np.ascontiguousarray(faces).astype(np.int64)
    P = np.ascontiguousarray(probabilities, dtype=np.float32)
    F = Fc.shape[0]

    pos = V[Fc]                                             # [F,3,3]
    bary = (pos[:, 0] + pos[:, 1] + pos[:, 2]) / np.float32(3.0)
    sq = (bary * bary).sum(-1, dtype=np.float32)

    bf = ml_dtypes.bfloat16
    bh = bary.astype(bf).astype(np.float32)
    bl = (bary - bh).astype(bf).astype(np.float32)
    sqh = sq.astype(bf).astype(np.float32)
    sql = (sq - sqh).astype(bf).astype(np.float32)

    rhs = np.zeros((KMM, FP), np.float32)
    rhs[0:3, :F] = (2.0 * bh).T
    rhs[3:6, :F] = (2.0 * bl).T
    rhs[6:9, :F] = (2.0 * bh).T
    rhs[9:12, :F] = (2.0 * bl).T
    rhs[12, :] = -1.0
    rhs[13, :] = -1.0
    rhs[14, :F] = -sqh
    rhs[15, :F] = -sql
    rhs[14, F:] = -1.0e30
    # band b at partitions [32b, 32b+16) holds candidates [b*GW, (b+1)*GW)
    rhs_bf = rhs.astype(bf)
    rhs_b = np.zeros((128, GW), bf)
    for b in range(NGRP):
        rhs_b[32 * b:32 * b + KMM] = rhs_bf[:, b * GW:(b + 1) * GW]

    lhsT = np.zeros((KMM, FP), np.float32)
    lhsT[0:3, :F] = bh.T
    lhsT[3:6, :F] = bh.T
    lhsT[6:9, :F] = bl.T
    lhsT[9:12, :F] = bl.T
    lhsT[12, :F] = sqh
    lhsT[13, :F] = sql
    lhsT[14, :] = 1.0
    lhsT[15, :] = 1.0
    lhsT_bf = lhsT.astype(bf)
    lhsT_b = np.zeros((128, FP), bf)
    for b in range(NGRP):
        lhsT_b[32 * b:32 * b + KMM] = lhsT_bf

    starts = pos[:, [0, 0, 1], :].reshape(F, 9)
    dirs = (pos[:, [1, 2, 2], :] - pos[:, [0, 0, 1], :]).reshape(F, 9)
    geo = np.zeros((FP, 18), np.float32)
    geo[:F, 0:9] = starts
    geo[:F, 9:18] = dirs

    probs_pad = np.zeros(FP, np.float32)
    probs_pad[:F] = P

    in1 = []
    for c in range(NCORES):
        lo, hi = c * NR, (c + 1) * NR
        in1.append({
            "lhsT": np.ascontiguousarray(lhsT_b[:, lo:hi]),
            "rhs": rhs_b,
        })
    aux = dict(F=F, geo=geo, probs_pad=probs_pad,
               bary=bary, sq=sq, bh=bh, bl=bl, sqh=sqh, sql=sql)
    return in1, aux


def _exact_rows_negd2(rows, aux):
    """Replicate the device -d2 rows in f32 (bf16-split products, f32 sums)."""
    bh, bl, sqh, sql = aux["bh"], aux["bl"], aux["sqh"], aux["sql"]
    F = aux["F"]
    rows = np.asarray(rows)
    live = rows < F                     # pad query rows have all-zero terms
    rc = np.where(live, rows, 0)
    S = len(rows)
    acc = np.zeros((S, FP), np.float32)
    for qp, cp in ((bh, bh), (bl, bh), (bh, bl), (bl, bl)):
        acc[:, :F] += (2 * qp[rc] * live[:, None]) @ cp.T
    acc[:, :F] -= ((sqh[rc] + sql[rc]) * live)[:, None]
    acc[:, :F] -= (sqh + sql)[None, :F]
    acc[:, F:] = -1.0e30
    return acc


def _host_merge(res1, aux):
    """Exact top-16 merge of per-chunk top-8s; returns nbr [FP, 16]."""
    vals = np.empty((FP, NC8), np.float32)
    lidx = np.empty((FP, NC8), np.uint16)
    for c in range(NCORES):
        vals[c * NR:(c + 1) * NR] = \
            np.asarray(res1.results[c]["cv"]).reshape(NR, NC8)
        lidx[c * NR:(c + 1) * NR] = \
            np.asarray(res1.results[c]["ci"]).reshape(NR, NC8)
    gidx = lidx.astype(np.int64) + \
        (np.arange(NC8, dtype=np.int64) // 8 * MXCH)[None, :]

    part = np.argpartition(-vals, KNN, axis=1)[:, :KNN]
    pv = np.take_along_axis(vals, part, axis=1)
    pg = np.take_along_axis(gidx, part, axis=1)
    order = np.lexsort((pg, -pv), axis=1)
    nbr = np.take_along_axis(pg, order, axis=1)             # [FP, 16]
    nv = np.take_along_axis(pv, order, axis=1)

    # truncation fallback: a chunk whose reported 8 values are all >= our
    # 16th could hide an unreported 9th that belongs in the top-16.
    F = aux["F"]
    v16 = nv[:, KNN - 1]
    chunk_min = vals[:, 7::8]                               # 8th value of each chunk
    suspect = np.nonzero((chunk_min >= v16[:, None]).any(1)
                         & (np.arange(FP) < F))[0]
    if suspect.size:
        negd2 = _exact_rows_negd2(suspect, aux)
        prt = np.argpartition(-negd2, KNN, axis=1)[:, :KNN]
        pvv = np.take_along_axis(negd2, prt, axis=1)
        o = np.lexsort((prt, -pvv), axis=1)
        nbr[suspect] = np.take_along_axis(prt, o, axis=1)
    return nbr


def _run(vertices, faces, probabilities, trace=False, **kw):
    p1, p2 = _get_progs()
    in1, aux = _host_prep(vertices, faces, probabilities)
    res1 = run_bass_kernel_spmd(p1, in1, list(range(NCORES)), trace=trace, **kw)
    nbr = _host_merge(res1, aux)                            # [FP, 16]
    F = aux["F"]

    geo = aux["geo"]
    geomN = geo[nbr]                                        # [FP, 16, 18]
    vp = (nbr != np.arange(FP)[:, None]).astype(np.float32) \
        * aux["probs_pad"][:, None]                         # [FP, 16]

    in2 = []
    for c in range(NCORES):
        lo, hi = c * NR, (c + 1) * NR
        in2.append({
            "geomN": np.ascontiguousarray(
                geomN[lo:hi].reshape(NT, 128, KNN, 18).transpose(1, 0, 2, 3)),
            "qgeom": np.ascontiguousarray(
                geo[lo:hi].reshape(NT, 128, 18).transpose(1, 0, 2)),
            "vp": np.ascontiguousarray(
                vp[lo:hi].reshape(NT, 128, KNN).transpose(1, 0, 2)),
        })
    res2 = run_bass_kernel_spmd(p2, in2, list(range(NCORES)), trace=trace, **kw)

    total = np.float64(0.0)
    for c in range(NCORES):
        total += np.asarray(res2.results[c]["wcross"], dtype=np.float64).sum()
    loss = np.float32(total / F)
    return loss, res1, res2, nbr


def run_device(vertices, faces, probabilities, trace=False, **kw):
    loss, res1, res2, _ = _run(vertices, faces, probabilities, trace=trace, **kw)
    return loss, (res1, res2)


def kernel(vertices, faces, probabilities):
    loss, *_ = _run(vertices, faces, probabilities)
    return np.array(loss, dtype=np.float32)



# revision 2
# speedup vs baseline: 3.0330x; 3.0330x over previous
"""EdgeCrossingsLoss Trainium2 kernel (8-core SPMD, data-parallel over query faces).

Windowed kNN: the host kd-sorts faces into 80 spatially-compact tiles of 128
queries and, per tile, selects the W=1280 candidates nearest the tile's
bounding box.  Each core then runs two device programs:

prog1 (per core, 10 tiles): PE computes -d2[q, c] for the tile's window via a
  16-row bf16 hi/lo-split matmul (exact products, f32 PSUM), ACT evacuates
  PSUM->SBUF, DVE takes top-8 values + in-chunk indices over 5 chunks of 256.
  Window coverage is guaranteed when the reported 16th distance is <= the
  window's point-to-box radius; rows violating that (or with a saturated
  chunk top-8) are recomputed exactly on the host (~2-3% of rows).

host: maps in-window indices to face ids, merges the 5 chunk-top-8s into the
  exact top-16 (value desc, index asc — the jax top_k tie-break), gathers the
  neighbor edge geometry, folds probabilities + self-mask into weights.

prog2 (per core): all 1280x16 3x3 line-line crossing tests as broadcast-AP
  tensor ops on DVE, hit = num^2 < EPS^2*|cross|^2, weight-masked, reduced.

Host sums the 8 per-core partials and divides by num_faces.
"""
import os
import numpy as np
import ml_dtypes
from contextlib import ExitStack

import concourse.bass as bass
import concourse.tile as tile
import concourse.bacc as bacc
from concourse import mybir
from concourse.bass_utils import run_bass_kernel_spmd

F32 = mybir.dt.float32
BF16 = mybir.dt.bfloat16
U16 = mybir.dt.uint16

NCORES = 8
KNN = 16
EPS = 1e-5
FP = 10240            # padded query count
NR = FP // NCORES     # 1280 query rows per core
NT = NR // 128        # 10 tiles of 128 rows per core
NTILES = FP // 128    # 80 tiles total
W = 1280              # candidate window per tile
NCH = 5               # top-8 chunks per window
CW = W // NCH         # 256-wide chunks
NC8 = NCH * 8         # 40 chunk-top-8 candidates per row
KMM = 16              # matmul contraction rows (bf16 hi/lo split)
MARGIN = 1e-3         # abs slack for device-vs-host value comparisons

ALU = mybir.AluOpType


def _build_prog1():
    nc = bacc.Bacc("TRN2", target_bir_lowering=False, debug=False,
                   num_devices=NCORES)
    lhsT_in = nc.dram_tensor("lhsT", [KMM, NR], BF16, kind="ExternalInput").ap()
    rhs_in = nc.dram_tensor("rhs", [KMM, NT, W], BF16, kind="ExternalInput").ap()
    cv_out = nc.dram_tensor("cv", [NT, 128, NC8], F32, kind="ExternalOutput").ap()
    ci_out = nc.dram_tensor("ci", [NT, 128, NC8], U16, kind="ExternalOutput").ap()

    with tile.TileContext(nc) as tc, ExitStack() as ctx:
        const_pool = ctx.enter_context(tc.tile_pool(name="const", bufs=1))
        rhs_pool = ctx.enter_context(tc.tile_pool(name="rhs", bufs=2))
        psum_pool = ctx.enter_context(tc.tile_pool(name="psum", bufs=3, space="PSUM"))
        negd2_pool = ctx.enter_context(tc.tile_pool(name="negd2", bufs=2))
        out_pool = ctx.enter_context(tc.tile_pool(name="out", bufs=2))

        lhsT_sb = const_pool.tile([KMM, NR], BF16)
        nc.sync.dma_start(lhsT_sb[:], lhsT_in[:])

        for t in range(NT):
            rhs_sb = rhs_pool.tile([KMM, W], BF16, tag="rhs")
            (nc.sync, nc.scalar)[t % 2].dma_start(rhs_sb[:], rhs_in[:, t])
            negd2 = negd2_pool.tile([128, W], F32, tag="negd2")
            cv = out_pool.tile([128, NC8], F32, tag="cv")
            ci = out_pool.tile([128, NC8], U16, tag="ci")
            for m in range(NCH):
                ps = psum_pool.tile([128, CW], F32, tag="ps")
                nc.tensor.matmul(
                    ps[:],
                    lhsT=lhsT_sb[:, t * 128:(t + 1) * 128],
                    rhs=rhs_sb[:, m * CW:(m + 1) * CW],
                    start=True, stop=True,
                )
                nc.scalar.copy(negd2[:, m * CW:(m + 1) * CW], ps[:])
                nc.vector.max(cv[:, m * 8:(m + 1) * 8],
                              negd2[:, m * CW:(m + 1) * CW])
                nc.vector.max_index(ci[:, m * 8:(m + 1) * 8],
                                    cv[:, m * 8:(m + 1) * 8],
                                    negd2[:, m * CW:(m + 1) * CW])
            nc.sync.dma_start(cv_out[t], cv[:])
            nc.scalar.dma_start(ci_out[t], ci[:])

    nc.compile()
    return nc


def _build_prog2():
    nc = bacc.Bacc("TRN2", target_bir_lowering=False, debug=False,
                   num_devices=NCORES)
    # host pre-transposes to partition-major layouts
    geom_in = nc.dram_tensor("geomN", [128, NT, KNN, 18], F32, kind="ExternalInput").ap()
    qgeom_in = nc.dram_tensor("qgeom", [128, NT, 18], F32, kind="ExternalInput").ap()
    vp_in = nc.dram_tensor("vp", [128, NT, KNN], F32, kind="ExternalInput").ap()
    wcross_out = nc.dram_tensor("wcross", [128, NT], F32, kind="ExternalOutput").ap()

    with tile.TileContext(nc) as tc, ExitStack() as ctx:
        pool = ctx.enter_context(tc.tile_pool(name="p", bufs=1))

        TS = NT * KNN
        # small inputs first so the ACT qgr replicate starts immediately;
        # geom as two large half-DMAs on separate HWDGE queues
        nc.sync.dma_start(qg := pool.tile([128, NT, 18], F32, name="qg"),
                          qgeom_in[:])
        nc.scalar.dma_start(vp := pool.tile([128, TS], F32, name="vp"),
                            vp_in[:].rearrange("p t s -> p (t s)"))
        geom = pool.tile([128, TS, 18], F32)
        H = NT // 2
        nc.sync.dma_start(
            geom[:, :H * KNN, :],
            geom_in[:, :H].rearrange("p t s c -> p (t s) c"))
        nc.scalar.dma_start(
            geom[:, H * KNN:, :],
            geom_in[:, H:].rearrange("p t s c -> p (t s) c"))

        # replicate query geometry per neighbor slot (ACT is otherwise idle)
        qgr = pool.tile([128, TS, 18], F32)
        nc.scalar.copy(
            qgr[:].rearrange("p (t s) c -> p t s c", t=NT),
            qg[:].unsqueeze(2).broadcast_to([128, NT, KNN, 18]))

        hit = pool.tile([128, TS, 3, 3], F32)

        def emit(beng, meng, x0, x1):
            """Edge tests for combined (tile, slot) range [x0, x1).
            beng runs the broadcast-AP ops (DVE); meng the unit-stride chain."""
            nx = x1 - x0
            SH = [128, nx, 3, 3]
            xsl = slice(x0, x1)

            def uc(c):   # query edge dir comp c (varies e1)
                return qgr[:, xsl, 9 + c:18:3].unsqueeze(3).broadcast_to(SH)

            def sc(c):   # query edge start comp c
                return qgr[:, xsl, c:9:3].unsqueeze(3).broadcast_to(SH)

            def vc(c):   # neighbor edge dir comp c (varies e2)
                return geom[:, xsl, 9 + c:18:3].unsqueeze(2).broadcast_to(SH)

            def tcp(c):  # neighbor edge start comp c
                return geom[:, xsl, c:9:3].unsqueeze(2).broadcast_to(SH)

            pfx = f"e{x0}"
            m = [pool.tile(SH, F32, name=f"{pfx}_m{i}") for i in range(6)]
            dif = [pool.tile(SH, F32, name=f"{pfx}_d{i}") for i in range(3)]
            cr = [pool.tile(SH, F32, name=f"{pfx}_cr{i}") for i in range(3)]
            BT = beng.tensor_tensor
            MT = meng.tensor_tensor
            for i in range(3):  # cr_i = u_{i+1} * v_{i+2} - u_{i+2} * v_{i+1}
                a, b = (i + 1) % 3, (i + 2) % 3
                BT(m[2 * i][:], uc(a), vc(b), ALU.mult)
                BT(m[2 * i + 1][:], uc(b), vc(a), ALU.mult)
            for c in range(3):
                BT(dif[c][:], tcp(c), sc(c), ALU.subtract)
            for i in range(3):
                MT(cr[i][:], m[2 * i][:], m[2 * i + 1][:], ALU.subtract)

            num = pool.tile(SH, F32, name=f"{pfx}_num")
            den2 = pool.tile(SH, F32, name=f"{pfx}_den2")
            t0 = pool.tile(SH, F32, name=f"{pfx}_t0")
            t1 = pool.tile(SH, F32, name=f"{pfx}_t1")
            MT(num[:], dif[0][:], cr[0][:], ALU.mult)
            MT(den2[:], cr[0][:], cr[0][:], ALU.mult)
            for c in (1, 2):
                MT(t0[:], dif[c][:], cr[c][:], ALU.mult)
                MT(num[:], num[:], t0[:], ALU.add)
                MT(t1[:], cr[c][:], cr[c][:], ALU.mult)
                MT(den2[:], den2[:], t1[:], ALU.add)
            MT(num[:], num[:], num[:], ALU.mult)          # num^2
            meng.tensor_scalar(den2[:], den2[:], float(EPS * EPS), None, ALU.mult)
            h = hit[:, xsl]
            MT(h, num[:], den2[:], ALU.is_lt)             # num^2 < eps^2*|cr|^2
            BT(h, h, vp[:, xsl].unsqueeze(2).unsqueeze(3).broadcast_to(SH),
               ALU.mult)

        emit(nc.vector, nc.vector, 0, TS // 2)
        emit(nc.vector, nc.vector, TS // 2, TS)

        wtile = pool.tile([128, NT], F32)
        nc.vector.tensor_reduce(
            wtile[:], hit[:].rearrange("p (t s) a b -> p t (s a b)", t=NT),
            mybir.AxisListType.X, ALU.add)

        nc.sync.dma_start(wcross_out[:], wtile[:])

    nc.compile()
    return nc


_PROGS = {}


def _get_progs():
    if "p1" not in _PROGS:
        _PROGS["p1"] = _build_prog1()
        _PROGS["p2"] = _build_prog2()
    return _PROGS["p1"], _PROGS["p2"]


def _kd_order(pts, tile=128):
    """Order point ids so that each consecutive block of `tile` ids is
    spatially compact (recursive median splits along the widest axis).
    len(pts) must be a multiple of `tile`."""
    out = []

    def rec(idx):
        if len(idx) <= tile:
            out.append(idx)
            return
        p = pts[idx]
        ax = int(np.argmax(p.max(0) - p.min(0)))
        srt = idx[np.argsort(p[:, ax], kind="stable")]
        nl = (len(idx) // tile // 2) * tile
        rec(srt[:nl])
        rec(srt[nl:])

    rec(np.arange(len(pts)))
    return np.concatenate(out)


def _host_prep(vertices, faces, probabilities):
    V = np.ascontiguousarray(vertices, dtype=np.float32)
    Fc = np.ascontiguousarray(faces).astype(np.int64)
    P = np.ascontiguousarray(probabilities, dtype=np.float32)
    F = Fc.shape[0]

    pos = V[Fc]                                             # [F,3,3]
    bary = (pos[:, 0] + pos[:, 1] + pos[:, 2]) / np.float32(3.0)
    sq = (bary * bary).sum(-1, dtype=np.float32)

    # spatially-compact query tiles (pad queries parked far away)
    bary_pad = np.full((FP, 3), 1.0e9, np.float32)
    bary_pad[:F] = bary
    order = _kd_order(bary_pad)                             # [FP]
    is_real = order < F

    bf = ml_dtypes.bfloat16
    bh = bary.astype(bf).astype(np.float32)
    bl = (bary - bh).astype(bf).astype(np.float32)
    sqh = sq.astype(bf).astype(np.float32)
    sql = (sq - sqh).astype(bf).astype(np.float32)

    # per-tile candidate windows: the W candidates nearest the tile's box
    # (point-to-box distance).  R2 = w-th distance => coverage radius.
    win = np.empty((NTILES, W), np.int64)
    R2 = np.empty(NTILES, np.float64)
    b64 = bary.astype(np.float64)
    for t in range(NTILES):
        rq = order[t * 128:(t + 1) * 128]
        rq = rq[rq < F]
        if rq.size == 0:
            win[t] = np.arange(W)
            R2[t] = -1.0
            continue
        box_lo = b64[rq].min(0)
        box_hi = b64[rq].max(0)
        d = np.maximum(box_lo - b64, 0.0) + np.maximum(b64 - box_hi, 0.0)
        dbox2 = (d * d).sum(1)
        ids = np.argpartition(dbox2, W - 1)[:W]
        R2[t] = dbox2[ids].max()
        # deterministic hash shuffle so spatial NN runs spread across chunks
        h = (ids.astype(np.uint64) * np.uint64(2654435761)) & np.uint64(0xFFFFFFFF)
        win[t] = ids[np.argsort(h, kind="stable")]

    # rhs: per-tile window candidate columns [KMM, NTILES, W]
    rhs_rows = np.zeros((KMM, F), np.float32)
    rhs_rows[0:3] = (2.0 * bh).T
    rhs_rows[3:6] = (2.0 * bl).T
    rhs_rows[6:9] = (2.0 * bh).T
    rhs_rows[9:12] = (2.0 * bl).T
    rhs_rows[12] = -1.0
    rhs_rows[13] = -1.0
    rhs_rows[14] = -sqh
    rhs_rows[15] = -sql
    rhs_all = rhs_rows.astype(bf)                           # [KMM, F]
    rhs_t = rhs_all[:, win.reshape(-1)].reshape(KMM, NTILES, W)

    # lhsT: per-query columns in kd-sorted order; pad queries -> zeros
    lhsT = np.zeros((KMM, FP), np.float32)
    oc = np.where(is_real, order, 0)
    lhsT[0:3] = np.where(is_real, bh[oc].T, 0.0)
    lhsT[3:6] = np.where(is_real, bh[oc].T, 0.0)
    lhsT[6:9] = np.where(is_real, bl[oc].T, 0.0)
    lhsT[9:12] = np.where(is_real, bl[oc].T, 0.0)
    lhsT[12] = np.where(is_real, sqh[oc], 0.0)
    lhsT[13] = np.where(is_real, sql[oc], 0.0)
    lhsT[14] = np.where(is_real, 1.0, 0.0)
    lhsT[15] = np.where(is_real, 1.0, 0.0)
    lhsT_bf = lhsT.astype(bf)

    starts = pos[:, [0, 0, 1], :].reshape(F, 9)
    dirs = (pos[:, [1, 2, 2], :] - pos[:, [0, 0, 1], :]).reshape(F, 9)
    geo = np.zeros((F, 18), np.float32)
    geo[:, 0:9] = starts
    geo[:, 9:18] = dirs

    in1 = []
    for c in range(NCORES):
        lo, hi = c * NR, (c + 1) * NR
        in1.append({
            "lhsT": np.ascontiguousarray(lhsT_bf[:, lo:hi]),
            "rhs": np.ascontiguousarray(rhs_t[:, c * NT:(c + 1) * NT]),
        })
    aux = dict(F=F, geo=geo, bary=bary, sq=sq, order=order, is_real=is_real,
               win=win, R2=R2, P=P)
    return in1, aux


def _exact_topk_rows(qids, aux):
    """Reference-style exact top-16 (f32 values, ties -> lowest face id)."""
    bary, sq, F = aux["bary"], aux["sq"], aux["F"]
    qb = bary[qids]                                         # [S,3]
    d2 = sq[qids][:, None] + sq[None, :] - 2.0 * (qb @ bary.T)
    part = np.argpartition(d2, KNN, axis=1)[:, :KNN]
    pv = np.take_along_axis(d2, part, axis=1)
    o = np.lexsort((part, pv), axis=1)
    return np.take_along_axis(part, o, axis=1)              # [S,16] face ids


def _host_merge(res1, aux):
    """Merge chunk-top-8s -> top-16 face ids per kd-sorted query row."""
    F, order, is_real = aux["F"], aux["order"], aux["is_real"]
    win, R2 = aux["win"], aux["R2"]

    vals = np.empty((FP, NC8), np.float32)
    lidx = np.empty((FP, NC8), np.int64)
    for c in range(NCORES):
        vals[c * NR:(c + 1) * NR] = \
            np.asarray(res1.results[c]["cv"]).reshape(NR, NC8)
        lidx[c * NR:(c + 1) * NR] = \
            np.asarray(res1.results[c]["ci"]).reshape(NR, NC8).astype(np.int64)
    tile_of = np.arange(FP) // 128
    wpos = (np.arange(NC8) // 8 * CW)[None, :] + lidx       # in-window position
    gidx = win[tile_of[:, None], wpos]                      # face ids [FP, NC8]

    part = np.argpartition(-vals, KNN, axis=1)[:, :KNN]
    pv = np.take_along_axis(vals, part, axis=1)
    pg = np.take_along_axis(gidx, part, axis=1)
    o = np.lexsort((pg, -pv), axis=1)
    nbr = np.take_along_axis(pg, o, axis=1)                 # [FP, 16]
    nv = np.take_along_axis(pv, o, axis=1)

    v16 = nv[:, KNN - 1]
    chunk8 = vals[:, 7::8]                                  # 8th value per chunk
    suspect = is_real & (
        (-v16 > R2[tile_of] - MARGIN)                       # coverage breach
        | (chunk8 >= (v16 - MARGIN)[:, None]).any(1))       # chunk truncation
    rows = np.nonzero(suspect)[0]
    if rows.size:
        nbr[rows] = _exact_topk_rows(order[rows], aux)
    return nbr


def _run(vertices, faces, probabilities, trace=False, **kw):
    p1, p2 = _get_progs()
    in1, aux = _host_prep(vertices, faces, probabilities)
    res1 = run_bass_kernel_spmd(p1, in1, list(range(NCORES)), trace=trace, **kw)
    nbr = _host_merge(res1, aux)                            # [FP, 16] face ids
    F = aux["F"]
    order, is_real = aux["order"], aux["is_real"]

    geo = aux["geo"]
    geomN = geo[np.minimum(nbr, F - 1)]                     # [FP, 16, 18]
    qgeo = np.zeros((FP, 18), np.float32)
    qgeo[is_real] = geo[order[is_real]]
    probs_sorted = np.zeros(FP, np.float32)
    probs_sorted[is_real] = aux["P"][order[is_real]]
    vp = (nbr != order[:, None]).astype(np.float32) * probs_sorted[:, None]

    in2 = []
    for c in range(NCORES):
        lo, hi = c * NR, (c + 1) * NR
        in2.append({
            "geomN": np.ascontiguousarray(
                geomN[lo:hi].reshape(NT, 128, KNN, 18).transpose(1, 0, 2, 3)),
            "qgeom": np.ascontiguousarray(
                qgeo[lo:hi].reshape(NT, 128, 18).transpose(1, 0, 2)),
            "vp": np.ascontiguousarray(
                vp[lo:hi].reshape(NT, 128, KNN).transpose(1, 0, 2)),
        })
    res2 = run_bass_kernel_spmd(p2, in2, list(range(NCORES)), trace=trace, **kw)

    total = np.float64(0.0)
    for c in range(NCORES):
        total += np.asarray(res2.results[c]["wcross"], dtype=np.float64).sum()
    loss = np.float32(total / F)
    return loss, res1, res2, nbr


def run_device(vertices, faces, probabilities, trace=False, **kw):
    loss, res1, res2, _ = _run(vertices, faces, probabilities, trace=trace, **kw)
    return loss, (res1, res2)


def kernel(vertices, faces, probabilities):
    loss, *_ = _run(vertices, faces, probabilities)
    return np.array(loss, dtype=np.float32)
